# revision 1
# baseline (speedup 1.0000x reference)
"""Multi-head self-attention with RoPE (causal) on 8 Trainium2 NeuronCores.

Sharding: core c -> batch b = c//4, head-group g = c%4 (heads 4g..4g+3).
Each core computes a partial output x[b] @ block of Wo; host sums the 4
partials per batch.

Per-core layout strategy (all matmuls in bf16, fp32 PSUM accumulation):
  - x is fed transposed (xT [1024, 2048]); q,k are produced directly in
    transposed layout qT/kT [256 dims, 2048 seq] (dims on partitions).
  - RoPE applied in transposed layout: pair-swap via DVE stream_shuffle,
    combine with host-precomputed cos / sign-folded sin tables.
  - scores computed transposed (scoresT [sk, sq]) so softmax's key-sum is a
    matmul reduction: a ones-column appended to v makes the PV matmul emit
    the softmax denominator as out row 64.
  - exp on ScalarE with scale=1/8 (the 1/sqrt(d_k)); causal masking by
    computing only sq >= 128*t_sk per key tile + one affine_select zeroing
    of the diagonal 128x128 block after exp.
"""

import ml_dtypes
import numpy as np

import concourse.bass as bass
import concourse.mybir as mybir
import concourse.tile as tile
from concourse import bacc
from concourse import library_config
from concourse.bass_utils import run_bass_kernel_spmd

F32 = mybir.dt.float32
F32R = mybir.dt.float32r
BF16 = mybir.dt.bfloat16

D = 1024          # d_model
NH = 16           # total heads
DK = 64           # head dim
S = 2048          # seq len
B = 2             # batch
THETA = 10000.0
HPC = 4           # heads per core
DPC = HPC * DK    # dims per core = 256
N_CORES = 8

SWAP_MASK = [(i ^ 1) for i in range(32)]  # pair-swap within 32-lane groups


def _mm(nc, out, lhsT, rhs, start, stop):
    nc.tensor.matmul(out, lhsT, rhs, start=start, stop=stop)


def _emit(tc, aps):
    nc = tc.nc
    xT, wq, wk, wv, wo, cosc, sinc, outp = (
        aps["xT"], aps["wqT"], aps["wkT"], aps["wvT"], aps["woT"],
        aps["cosT"], aps["sinT"], aps["out"],
    )
    AF = mybir.ActivationFunctionType
    OP = mybir.AluOpType

    with (
        tc.tile_pool(name="persist", bufs=1) as pp,
        tc.tile_pool(name="ropetmp", bufs=4) as rt,
        tc.tile_pool(name="attn", bufs=2) as pa,
        tc.tile_pool(name="exp", bufs=14) as pe,
        tc.tile_pool(name="psum", bufs=2, space="PSUM") as psa,
    ):
        # ---- persistent SBUF tensors ----
        qT_sb = pp.tile([128, 2, S], BF16, tag="qT")
        kT_sb = pp.tile([128, 2, S], BF16, tag="kT")
        v_sb = pp.tile([128, 16, HPC, DK + 1], BF16, tag="v")
        wo_sb = pp.tile([128, 2, D], BF16, tag="wo")
        attnT_sb = pp.tile([128, 2, S], BF16, tag="attnT")
        dmask_sb = pp.tile([128, 128], BF16, tag="dmask")
        xT_sb = pp.tile([128, 8, S], BF16, tag="xT")
        wq_sb = pp.tile([128, 8, DPC], BF16, tag="wq")
        wk_sb = pp.tile([128, 8, DPC], BF16, tag="wk")
        wv_sb = pp.tile([128, 8, DPC], BF16, tag="wv")
        cos_sb = pp.tile([128, S], F32, tag="cos")
        sin_sb = pp.tile([128, S], F32, tag="sin")

        # input DMAs, ordered so the first projection chunk unblocks ASAP:
        # wq fully, then xT's first 512 columns across all k-tiles
        def xT_dma(c):
            for kt in range(8):
                nc.sync.dma_start(
                    xT_sb[:, kt, 512 * c:512 * (c + 1)],
                    xT[128 * kt:128 * (kt + 1), 512 * c:512 * (c + 1)])
        def cs_dma(c):
            sl = slice(512 * c, 512 * (c + 1))
            nc.sync.dma_start(cos_sb[:, sl], cosc[:, sl])
            nc.sync.dma_start(sin_sb[:, sl], sinc[:, sl])
        for kt in range(8):
            nc.sync.dma_start(wq_sb[:, kt, :], wq[128 * kt:128 * (kt + 1), :])
        xT_dma(0)
        cs_dma(0)
        for kt in range(8):
            nc.sync.dma_start(wk_sb[:, kt, :], wk[128 * kt:128 * (kt + 1), :])
        xT_dma(1)
        cs_dma(1)
        xT_dma(2)
        cs_dma(2)
        xT_dma(3)
        cs_dma(3)
        nc.sync.dma_start(dmask_sb[:], aps["dmask"][:])
        for kt in range(8):
            nc.sync.dma_start(wv_sb[:, kt, :], wv[128 * kt:128 * (kt + 1), :])
        nc.sync.dma_start(wo_sb[:], wo.rearrange("(n p) m -> p n m", p=128))
        # ones column of v (denominator trick)
        nc.gpsimd.memset(v_sb[:, :, :, DK], 1.0)
        ones_sb = pp.tile([DK + 1, DK], BF16, tag="ones")
        nc.gpsimd.memset(ones_sb[:], 1.0)

        def qk_chunk(w_sb, outT, mt, c):
                ps = psa.tile([128, 512], F32, tag="pj", bufs=2)
                for kt in range(8):
                    _mm(nc, ps[:],
                        w_sb[:, kt, 128 * mt:128 * (mt + 1)],
                        xT_sb[:, kt, 512 * c:512 * (c + 1)],
                        start=(kt == 0), stop=(kt == 7))
                sl = slice(512 * c, 512 * (c + 1))
                sw = rt.tile([128, 512], F32, tag="sw")
                nc.vector.stream_shuffle(sw[:], ps[:], SWAP_MASK)
                t1 = rt.tile([128, 512], BF16, tag="t1")
                nc.vector.tensor_tensor(t1[:], ps[:], cos_sb[:, sl], OP.mult)
                t2 = rt.tile([128, 512], BF16, tag="t2")
                nc.gpsimd.tensor_tensor(t2[:], sw[:], sin_sb[:, sl], OP.mult)
                nc.vector.tensor_tensor(outT[:, mt, sl], t1[:], t2[:], OP.add)

        def v_proj():
            for st2 in range(8):
                ps = psa.tile([128, 512], F32, tag="pj", bufs=2)
                for half in range(2):
                    st = 2 * st2 + half
                    for kt in range(8):
                        _mm(nc, ps[:, DPC * half:DPC * (half + 1)],
                            xT_sb[:, kt, 128 * st:128 * (st + 1)],
                            wv_sb[:, kt, :],
                            start=(kt == 0), stop=(kt == 7))
                nc.vector.tensor_copy(
                    v_sb[:, 2 * st2:2 * st2 + 2, :, 0:DK],
                    ps[:].rearrange("p (s h e) -> p s h e", s=2, h=HPC),
                )

        def attention(h):
            sub, ph = h % 2, h // 2
            prow = slice(64 * sub, 64 * (sub + 1))
            for H in range(2):
                q_hi = 1024 * (H + 1)
                at_h = [psa.tile([DK + 1, 512], F32, tag="at", bufs=2,
                                 name=f"at{H}{bh}")
                        for bh in range(2)]
                t_hi = 8 * (H + 1)  # exclusive
                for t in range(t_hi):
                    sq_lo = max(128 * t, 1024 * H)
                    L = q_hi - sq_lo
                    sc = psa.tile([128, 1024], F32, tag="sc", bufs=2)
                    off = 0
                    while off < L:
                        n = min(512, L - off)
                        _mm(nc, sc[:, off:off + n],
                            kT_sb[prow, ph, 128 * t:128 * (t + 1)],
                            qT_sb[prow, ph, sq_lo + off:sq_lo + off + n],
                            start=True, stop=True)
                        off += n
                    ex = pe.tile([128, 1024], BF16, tag="exp")
                    nc.scalar.activation(ex[:, 0:L], sc[:, 0:L], AF.Exp, scale=0.125)
                    if 128 * t >= 1024 * H:
                        # diagonal block: zero exp where local_sq < partition
                        nc.vector.tensor_tensor(
                            ex[:, 0:128], ex[:, 0:128], dmask_sb[:], OP.mult)
                    # PV accumulation (+ denominator via ones column)
                    for ck in range(2):
                        c_lo, c_hi = 1024 * H + 512 * ck, 1024 * H + 512 * (ck + 1)
                        if sq_lo >= c_hi:
                            continue
                        lo = max(sq_lo, c_lo)
                        last_t = min(t_hi, (c_hi + 127) // 128) - 1
                        _mm(nc, at_h[ck][:, lo - c_lo:512],
                            v_sb[:, t, h, :],
                            ex[:, lo - sq_lo:c_hi - sq_lo],
                            start=(t == 0), stop=(t == last_t))
                # normalize, pipelined in 512-col halves: each half's
                # recip/copy/broadcast/multiply starts as soon as that
                # half's last PV lands (chunk A finishes before chunk B)
                rc = pa.tile([DK + 1, 1024], BF16, tag="rc")
                ac = pa.tile([DK, 1024], BF16, tag="ac")
                bc_ps = psa.tile([DK, 1024], F32, tag="sc", bufs=2)
                tn = pa.tile([DK, 1024], BF16, tag="tn")
                for bh in range(2):
                    hsl = slice(512 * bh, 512 * (bh + 1))
                    with nc.allow_low_precision(reason="bf16 softmax recip"):
                        nc.vector.reciprocal(
                            rc[DK:DK + 1, hsl], at_h[bh][DK:DK + 1, :])
                    nc.scalar.copy(ac[:, hsl], at_h[bh][0:DK, :])
                    _mm(nc, bc_ps[:, hsl],
                        ones_sb[DK:DK + 1, :], rc[DK:DK + 1, hsl],
                        start=True, stop=True)
                    osl = slice(1024 * H + 512 * bh, 1024 * H + 512 * (bh + 1))
                    if sub == 0:
                        nc.vector.tensor_tensor(
                            attnT_sb[0:64, ph, osl],
                            ac[:, hsl], bc_ps[:, hsl], OP.mult)
                    else:
                        nc.vector.tensor_tensor(
                            tn[:, hsl], ac[:, hsl], bc_ps[:, hsl], OP.mult)
                        nc.sync.dma_start(
                            attnT_sb[64:128, ph, osl], tn[:, hsl])

        def out_proj():
            optags = [("pj", 2), ("sc", 2), ("at", 2)]
            for st in range(16):
                ob = pa.tile([128, 1024], BF16, tag="ob", bufs=4)
                for ncb in range(2):
                    tg, bf = optags[(2 * st + ncb) % 3]
                    po = psa.tile([128, 512], F32, tag=tg, bufs=bf)
                    for kt2 in range(2):
                        _mm(nc, po[:, 0:512],
                            attnT_sb[:, kt2, 128 * st:128 * (st + 1)],
                            wo_sb[:, kt2, 512 * ncb:512 * (ncb + 1)],
                            start=(kt2 == 0), stop=(kt2 == 1))
                    if ncb == 0:
                        nc.scalar.copy(ob[:, 0:512], po[:, 0:512])
                    else:
                        nc.vector.tensor_copy(ob[:, 512:1024], po[:, 0:512])
                nc.sync.dma_start(
                    outp[128 * st:128 * (st + 1), :], ob[:])

        # head-pair pipelined emission: attention on heads 0,1 overlaps
        # the projections for heads 2,3
        for c in range(4):
            qk_chunk(wq_sb, qT_sb, 0, c)
            qk_chunk(wk_sb, kT_sb, 0, c)
        v_proj()
        with tc.high_priority():
            attention(0)
            attention(1)
        for c in range(4):
            qk_chunk(wq_sb, qT_sb, 1, c)
            qk_chunk(wk_sb, kT_sb, 1, c)
        attention(3)
        attention(2)
        out_proj()


_CACHE = {}


def _build():
    if "nc" in _CACHE:
        return _CACHE["nc"], _CACHE["aps"]
    nc = bacc.Bacc("TRN2", target_bir_lowering=False, debug=False,
                   enable_asserts=False, num_devices=N_CORES)
    aps = {
        "xT": nc.dram_tensor("xT", [D, S], BF16, kind="ExternalInput").ap(),
        "wqT": nc.dram_tensor("wqT", [D, DPC], BF16, kind="ExternalInput").ap(),
        "wkT": nc.dram_tensor("wkT", [D, DPC], BF16, kind="ExternalInput").ap(),
        "wvT": nc.dram_tensor("wvT", [D, DPC], BF16, kind="ExternalInput").ap(),
        "woT": nc.dram_tensor("woT", [DPC, D], BF16, kind="ExternalInput").ap(),
        "cosT": nc.dram_tensor("cosT", [128, S], F32, kind="ExternalInput").ap(),
        "sinT": nc.dram_tensor("sinT", [128, S], F32, kind="ExternalInput").ap(),
        "dmask": nc.dram_tensor("dmask", [128, 128], BF16, kind="ExternalInput").ap(),
        "out": nc.dram_tensor("out", [S, D], BF16, kind="ExternalOutput").ap(),
    }
    with tile.TileContext(nc) as tc:
        _emit(tc, aps)
    nc.compile()
    _CACHE["nc"], _CACHE["aps"] = nc, aps
    return nc, aps


def _host_tables():
    pos = np.arange(S, dtype=np.float64)
    freqs = THETA ** (-np.arange(0, DK, 2, dtype=np.float64) / DK)
    ang = pos[:, None] * freqs[None, :]          # [S, 32]
    cos64 = np.empty((64, S), np.float32)
    sin64 = np.empty((64, S), np.float32)
    cos64[0::2] = cos64[1::2] = np.cos(ang).T
    sin64[0::2] = -np.sin(ang).T
    sin64[1::2] = np.sin(ang).T
    return (np.ascontiguousarray(np.concatenate([cos64, cos64], axis=0)),
            np.ascontiguousarray(np.concatenate([sin64, sin64], axis=0)))


def make_in_maps(x, Wq, Wk, Wv, Wo):
    cosT, sinT = _host_tables()
    dmask = np.triu(np.ones((128, 128), ml_dtypes.bfloat16))  # keep sq >= sk
    xT = [np.ascontiguousarray(x[b].T.astype(ml_dtypes.bfloat16)) for b in range(B)]
    maps = []
    for c in range(N_CORES):
        b, g = c // 4, c % 4
        rows = slice(DPC * g, DPC * (g + 1))
        maps.append({
            "xT": xT[b],
            "wqT": np.ascontiguousarray(Wq[rows, :].T.astype(ml_dtypes.bfloat16)),
            "wkT": np.ascontiguousarray(Wk[rows, :].T.astype(ml_dtypes.bfloat16)),
            "wvT": np.ascontiguousarray(Wv[rows, :].T.astype(ml_dtypes.bfloat16)),
            "woT": np.ascontiguousarray(Wo[:, rows].T.astype(ml_dtypes.bfloat16)),
            "cosT": cosT,
            "sinT": sinT,
            "dmask": dmask,
        })
    return maps


def kernel(x, Wq, Wk, Wv, Wo, _trace=False, _tmpdir=None):
    x, Wq, Wk, Wv, Wo = (np.asarray(a, dtype=np.float32) for a in (x, Wq, Wk, Wv, Wo))
    nc, _ = _build()
    maps = make_in_maps(x, Wq, Wk, Wv, Wo)
    res = run_bass_kernel_spmd(nc, maps, core_ids=list(range(N_CORES)),
                               trace=_trace, tmpdir=_tmpdir)
    out = np.zeros((B, S, D), np.float32)
    for c in range(N_CORES):
        out[c // 4] += res.results[c]["out"].astype(np.float32)
    if _trace:
        kernel.last_results = res
    return out



# revision 11
# speedup vs baseline: 1.0503x; 1.0503x over previous
"""Multi-head self-attention with RoPE (causal) on 8 Trainium2 NeuronCores.

Sharding: core c -> batch b = c//4, head-group g = c%4 (heads 4g..4g+3).
Each core computes a partial output x[b] @ block of Wo; host sums the 4
partials per batch (and divides by the fp8 weight scaling).

v2 design (cost-model driven):
  - q/k/v projections and scores in fp8-e4m3 DoubleRow matmuls (2 K-slabs
    per instruction, 0.5 cyc/row): weights are host-scaled x16 and
    row-permuted so each head's q/k lives as [32 partitions, 2 slabs] for
    the DoubleRow score contraction over d_k=64.
  - RoPE pair-swap comes from a second projection with host-swapped weight
    rows (PE work instead of DVE stream_shuffle); cos/sin tables carry the
    1/16 dequant scale.
  - exp is split between ScalarE (native Exp) and DVE (Schraudolph bit-hack
    exp: int16(x*A+B) bitcast to bf16), balanced by a running load counter.
  - PV uses the flipped layout out[sq, dk] (queries on partitions):
    softmax denominator becomes a per-partition scalar (cheap normalize via
    activation-scale), accumulated via N=1 ones-matmuls.
  - attn tiles are PE-transposed back to [dk, sq] (fp8) for a DoubleRow
    out-projection; out-proj PSUM is DMA'd to DRAM as f32 directly
    (host divides by the 64x weight scaling).
"""

import ml_dtypes
import numpy as np

import concourse.bass as bass
import concourse.mybir as mybir
import concourse.tile as tile
from concourse import bacc
from concourse.bass_utils import run_bass_kernel_spmd

F32 = mybir.dt.float32
BF16 = mybir.dt.bfloat16
I16 = mybir.dt.int16
FP8 = mybir.dt.float8e4
DR = mybir.MatmulPerfMode.DoubleRow

D = 1024          # d_model
NH = 16           # total heads
DK = 64           # head dim
S = 2048          # seq len
B = 2             # batch
THETA = 10000.0
HPC = 4           # heads per core
DPC = HPC * DK    # dims per core = 256
N_CORES = 8

WS = 16.0                    # fp8 weight scale (all W * 16)
ATS = 4.0                    # attn tile scale before fp8 out-proj
OUT_DIV = WS * ATS           # host divides final output by this
LOG2E = 1.4426950408889634
EXP_A = 0.125 * 128.0 * LOG2E      # schraudolph mult (incl 1/sqrt(dk))
EXP_B = 127.0 * 128.0 - 5.5        # schraudolph bias (tuned C=-5.5)
QHI = 512                          # rows < QHI use the bf16 hi-precision path
SWAP_MASK = [(i ^ 1) for i in range(32)]


def _emit(tc, aps):
    nc = tc.nc
    OP = mybir.AluOpType
    AF = mybir.ActivationFunctionType

    load = {"act": 0.0, "dve": 0.0}

    with (
        tc.tile_pool(name="persist", bufs=1) as pp,
        tc.tile_pool(name="rope", bufs=5) as rp,
        tc.tile_pool(name="expp", bufs=6) as xp,
        tc.tile_pool(name="small", bufs=6) as sm,
        tc.tile_pool(name="ps512", bufs=4, space="PSUM") as ps4,
        tc.tile_pool(name="psacc", bufs=1, space="PSUM") as psA,
    ):
        xT_sb = pp.tile([128, 4, 2, S], FP8, tag="xT8")
        wq_sb = pp.tile([128, 4, 2, 2, 128], FP8, tag="wq8")
        wk_sb = pp.tile([128, 4, 2, 2, 128], FP8, tag="wk8")
        wqs_sb = pp.tile([128, 4, 2, 2, 128], FP8, tag="wq8s")
        wks_sb = pp.tile([128, 4, 2, 2, 128], FP8, tag="wk8s")
        wv_sb = pp.tile([128, 4, 2, DPC], FP8, tag="wv8")
        wo_sb = pp.tile([128, 2, D], FP8, tag="wo8")
        cos_sb = pp.tile([128, 2, S], BF16, tag="cos")
        sin_sb = pp.tile([128, 2, S], BF16, tag="sin")
        qT8 = pp.tile([128, 2, S], FP8, tag="qT8")
        kT8 = pp.tile([128, 2, S], FP8, tag="kT8")
        v_sb = pp.tile([128, 16, HPC, DK], BF16, tag="v")
        attnT8 = pp.tile([128, 2, S], FP8, tag="attnT8")
        id_sb = pp.tile([128, 128], BF16, tag="ident")
        dm_sb = pp.tile([128, 128], BF16, tag="dmask")
        ones_sb = pp.tile([128, 1], BF16, tag="ones")
        recip_sb = pp.tile([128, 2, 16], F32, tag="recip")
        xT_hi = pp.tile([128, 8, QHI], BF16, tag="xT_hi")
        wq_hi = pp.tile([128, 8, 2, 128], BF16, tag="wq_hi")
        wk_hi = pp.tile([128, 8, 2, 128], BF16, tag="wk_hi")
        wv_hi = pp.tile([128, 8, DPC], BF16, tag="wv_hi")
        wo_hi = pp.tile([128, 2, D], BF16, tag="wo_hi")
        qT_hi = pp.tile([128, 2, QHI], BF16, tag="qT_hi")
        kT_hi = pp.tile([128, 2, QHI], BF16, tag="kT_hi")
        v_hi = pp.tile([128, 4, HPC, DK], BF16, tag="v_hi")
        attnT_hi = pp.tile([128, 2, QHI], BF16, tag="attnT_hi")

        at_ps = psA.tile([128, 16, DK], F32, tag="at")     # 2 banks
        den_ps = psA.tile([128, 16, 1], F32, tag="den")    # 1 bank
        trp_ps = psA.tile([128, 2, 128], BF16, tag="trp")  # 1 bank

        # ---- input DMAs, ordered to unblock the q-projection first ----
        dma = nc.sync.dma_start
        dma(wq_sb[:], aps["wq8"][:])
        dma(wqs_sb[:], aps["wq8s"][:])
        dma(xT_sb[:, :, :, 0:512], aps["xT8"][:, :, :, 0:512])
        dma(cos_sb[:, :, 0:1024], aps["cosT"][:, :, 0:1024])
        dma(sin_sb[:, :, 0:1024], aps["sinT"][:, :, 0:1024])
        dma(xT_sb[:, :, :, 512:1024], aps["xT8"][:, :, :, 512:1024])
        dma(wk_sb[:], aps["wk8"][:])
        dma(wks_sb[:], aps["wk8s"][:])
        dma(xT_sb[:, :, :, 1024:1536], aps["xT8"][:, :, :, 1024:1536])
        dma(cos_sb[:, :, 1024:2048], aps["cosT"][:, :, 1024:2048])
        dma(sin_sb[:, :, 1024:2048], aps["sinT"][:, :, 1024:2048])
        dma(xT_sb[:, :, :, 1536:2048], aps["xT8"][:, :, :, 1536:2048])
        dma(wv_sb[:], aps["wv8"][:])
        dma(id_sb[:], aps["ident"][:])
        dma(dm_sb[:], aps["dmask"][:])
        dma(wo_sb[:], aps["wo8"][:])
        dma(xT_hi[:], aps["xT_hi"][:])
        dma(wq_hi[:], aps["wq_hi"][:])
        dma(wk_hi[:], aps["wk_hi"][:])
        dma(wv_hi[:], aps["wv_hi"][:])
        dma(wo_hi[:], aps["wo_hi"][:])
        nc.gpsimd.memset(ones_sb[:], 1.0)

        # ---- projections (fp8 DoubleRow) + RoPE ----
        def dr_proj(w_sb, mt, sl):
            ps = ps4.tile([128, 512], F32, tag="ps512")
            for kt2 in range(4):
                nc.tensor.matmul(ps[:], w_sb[:, kt2, :, mt, :],
                                 xT_sb[:, kt2, :, sl],
                                 start=(kt2 == 0), stop=(kt2 == 3),
                                 perf_mode=DR)
            return ps

        def rope_chunk(wn, wsw, outT, mt, c):
            sl = slice(512 * c, 512 * (c + 1))
            ps = dr_proj(wn, mt, sl)
            pss = dr_proj(wsw, mt, sl)
            t1 = rp.tile([128, 512], BF16, tag="t1")
            nc.vector.tensor_tensor(t1[:], ps[:], cos_sb[:, mt, sl], OP.mult)
            load["dve"] += 660
            qs = rp.tile([128, 512], BF16, tag="qs")
            nc.scalar.copy(qs[:], pss[:])
            load["act"] += 570
            t2 = rp.tile([128, 512], BF16, tag="t2")
            nc.gpsimd.tensor_tensor(t2[:], qs[:], sin_sb[:, mt, sl], OP.mult)
            with nc.allow_low_precision(reason="fp8 qk write"):
                nc.vector.tensor_tensor(outT[:, mt, sl], t1[:], t2[:], OP.add)
            load["dve"] += 600

        def v_chunk(st):
            ps = ps4.tile([128, 512], F32, tag="ps512")
            for kt2 in range(4):
                nc.tensor.matmul(ps[:, 0:DPC],
                                 xT_sb[:, kt2, :, 128 * st:128 * (st + 1)],
                                 wv_sb[:, kt2, :, :],
                                 start=(kt2 == 0), stop=(kt2 == 3),
                                 perf_mode=DR)
            with nc.allow_low_precision(reason="bf16 v"):
                nc.scalar.mul(v_sb[:, st, :, :],
                              ps[:, 0:DPC].rearrange("p (h e) -> p h e", h=HPC),
                              1.0 / WS)
            load["act"] += 360

        for c in range(4):
            rope_chunk(wq_sb, wqs_sb, qT8, 0, c)
            rope_chunk(wq_sb, wqs_sb, qT8, 1, c)
        rope_chunk(wk_sb, wks_sb, kT8, 0, 0)
        rope_chunk(wk_sb, wks_sb, kT8, 1, 0)
        for st in range(0, 4):
            v_chunk(st)
        for c in range(1, 4):
            rope_chunk(wk_sb, wks_sb, kT8, 0, c)
            rope_chunk(wk_sb, wks_sb, kT8, 1, c)
            for st in range(4 * c, 4 * c + 4):
                v_chunk(st)

        # ---- hi-precision (bf16) projections + rope for rows < QHI ----
        def hi_rope(w_hi, outT, mt):
            ps = ps4.tile([128, 512], F32, tag="ps512")
            for kt in range(8):
                nc.tensor.matmul(ps[:], w_hi[:, kt, mt, :], xT_hi[:, kt, :],
                                 start=(kt == 0), stop=(kt == 7))
            sw = rp.tile([128, 512], F32, tag="sw")
            nc.vector.stream_shuffle(sw[:], ps[:], SWAP_MASK)
            load["dve"] += 660
            t1 = rp.tile([128, 512], BF16, tag="t1")
            nc.vector.tensor_tensor(t1[:], ps[:], cos_sb[:, mt, 0:QHI], OP.mult)
            load["dve"] += 660
            t2 = rp.tile([128, 512], BF16, tag="t2")
            nc.gpsimd.tensor_tensor(t2[:], sw[:], sin_sb[:, mt, 0:QHI], OP.mult)
            with nc.allow_low_precision(reason="bf16 hi qk"):
                nc.vector.tensor_tensor(outT[:, mt, :], t1[:], t2[:], OP.add)
            load["dve"] += 200

        def hi_v(st):
            ps = ps4.tile([128, 512], F32, tag="ps512")
            for kt in range(8):
                nc.tensor.matmul(ps[:, 0:DPC],
                                 xT_hi[:, kt, 128 * st:128 * (st + 1)],
                                 wv_hi[:, kt, :],
                                 start=(kt == 0), stop=(kt == 7))
            with nc.allow_low_precision(reason="bf16 v hi"):
                nc.scalar.mul(v_hi[:, st, :, :],
                              ps[:, 0:DPC].rearrange("p (h e) -> p h e", h=HPC),
                              1.0 / WS)
            load["act"] += 360

        for mt in range(2):
            hi_rope(wq_hi, qT_hi, mt)
            hi_rope(wk_hi, kT_hi, mt)
        for st in range(4):
            hi_v(st)

        # ---- attention (sequential heads, flipped PV) ----
        def exp_chunk(sc, ex, n, diag):
            if load["act"] <= load["dve"]:
                nc.scalar.activation(ex[:, 0:n], sc[:, 0:n], AF.Exp, scale=0.125)
                load["act"] += n * 0.833 + 170
            else:
                with nc.allow_low_precision(reason="schraudolph exp"):
                    nc.vector.tensor_scalar(ex[:, 0:n].bitcast(I16), sc[:, 0:n],
                                            EXP_A, EXP_B, OP.mult, OP.add)
                load["dve"] += n * 1.042 + 170
            if diag:
                nc.gpsimd.tensor_tensor(ex[:, 0:128], ex[:, 0:128], dm_sb[:],
                                        OP.mult)

        def norm_one(h, qt):
            a_t = sm.tile([128, DK], BF16, tag="attn")
            rc = recip_sb[:, h % 2, qt:qt + 1]
            if load["act"] <= load["dve"]:
                with nc.allow_low_precision(reason="bf16 attn"):
                    nc.scalar.mul(a_t[:], at_ps[:, qt, :], rc)
                load["act"] += 200
            else:
                with nc.allow_low_precision(reason="bf16 attn"):
                    nc.vector.tensor_scalar(a_t[:], at_ps[:, qt, :], rc, None,
                                            OP.mult)
                load["dve"] += 200
            prow = slice(64 * (h % 2), 64 * (h % 2) + 64)
            nc.tensor.transpose(trp_ps[prow, qt % 2, :], a_t[:], id_sb[:])
            if qt < QHI // 128:
                dst = attnT_hi[prow, h // 2, 128 * qt:128 * (qt + 1)]
                scl = 1.0
            else:
                dst = attnT8[prow, h // 2, 128 * qt:128 * (qt + 1)]
                scl = ATS
            with nc.allow_low_precision(reason="fp8 attnT"):
                if load["act"] <= load["dve"]:
                    nc.scalar.mul(dst, trp_ps[prow, qt % 2, :], scl)
                    load["act"] += 260
                else:
                    nc.vector.tensor_scalar(dst, trp_ps[prow, qt % 2, :], scl,
                                            None, OP.mult)
                    load["dve"] += 260

        def out_st(st, ob):
            hi = st < QHI // 128
            for ncb in range(2):
                po = ps4.tile([128, 512], F32, tag="ps512")
                if hi:
                    for kt2 in range(2):
                        nc.tensor.matmul(
                            po[:], attnT_hi[:, kt2, 128 * st:128 * (st + 1)],
                            wo_hi[:, kt2, 512 * ncb:512 * (ncb + 1)],
                            start=(kt2 == 0), stop=(kt2 == 1))
                else:
                    nc.tensor.matmul(po[:],
                                     attnT8[:, :, 128 * st:128 * (st + 1)],
                                     wo_sb[:, :, 512 * ncb:512 * (ncb + 1)],
                                     start=True, stop=True, perf_mode=DR)
                osc = (1.0 / WS) if hi else (1.0 / OUT_DIV)
                with nc.allow_low_precision(reason="bf16 out"):
                    if load["act"] <= load["dve"]:
                        nc.scalar.mul(ob[:, st % 2, ncb, :], po[:], osc)
                        load["act"] += 580
                    else:
                        nc.vector.tensor_scalar(ob[:, st % 2, ncb, :], po[:],
                                                osc, None, OP.mult)
                        load["dve"] += 670
            if st % 2 == 1:
                dst = aps["out"][256 * (st // 2):256 * (st // 2 + 1), :]
                dma(dst.rearrange("(s p) (n c) -> p s n c", s=2, n=2), ob[:])

        def finish_group(h, qg):
            qsl = slice(4 * qg, 4 * qg + 4)
            with nc.allow_low_precision(reason="recip"):
                nc.vector.reciprocal(recip_sb[:, h % 2, qsl],
                                     den_ps[:, qsl, 0])
            load["dve"] += 180
            ob = None
            for qt in range(4 * qg, 4 * qg + 4):
                norm_one(h, qt)
                if h == 3:
                    if qt % 2 == 0:
                        ob = sm.tile([128, 2, 2, 512], BF16, tag="ob", bufs=2)
                    out_st(qt, ob)

        def attention(h):
            hp = slice(32 * h, 32 * h + 32)
            # hi-precision prologue: queries < QHI, keys < QHI (bf16)
            for t in range(QHI // 128):
                base = 128 * t
                n = QHI - base
                sc = ps4.tile([128, 512], F32, tag="ps512")
                for mt in range(2):
                    nc.tensor.matmul(sc[:, 0:n], kT_hi[hp, mt, base:base + 128],
                                     qT_hi[hp, mt, base:QHI],
                                     start=(mt == 0), stop=(mt == 1),
                                     tile_position=(32 * h, 0))
                ex = xp.tile([128, 512], BF16, tag="exp")
                nc.scalar.activation(ex[:, 0:n], sc[:, 0:n], AF.Exp, scale=0.125)
                load["act"] += n * 0.833 + 170
                nc.gpsimd.tensor_tensor(ex[:, 0:128], ex[:, 0:128], dm_sb[:],
                                        OP.mult)
                for j in range(n // 128):
                    qt = t + j
                    exj = ex[:, 128 * j:128 * (j + 1)]
                    nc.tensor.matmul(at_ps[:, qt, :], exj, v_hi[:, t, h, :],
                                     start=(t == 0 and qt == 0), stop=(t == qt),
                                     skip_group_check=True)
                    nc.tensor.matmul(den_ps[:, qt, :], exj, ones_sb[:],
                                     start=(t == 0 and qt == 0), stop=(t == qt),
                                     skip_group_check=True)
                if t == 3:
                    finish_group(h, 0)
            for t in range(16):
                base = 128 * t
                L = S - base
                off = max(0, QHI - base)
                while off < L:
                    n = min(512, L - off)
                    sc = ps4.tile([128, 512], F32, tag="ps512")
                    nc.tensor.matmul(sc[:, 0:n], kT8[hp, :, base:base + 128],
                                     qT8[hp, :, base + off:base + off + n],
                                     start=True, stop=True, perf_mode=DR,
                                     tile_position=(32 * h, 0))
                    ex = xp.tile([128, 512], BF16, tag="exp")
                    exp_chunk(sc, ex, n, diag=(off == 0))
                    for j in range(n // 128):
                        qt = t + (off // 128) + j
                        exj = ex[:, 128 * j:128 * (j + 1)]
                        # start=True zeroes the whole 2KB PSUM bank (lazily,
                        # realized on write) -> only the first write of each
                        # bank per accumulation round may carry it.
                        nc.tensor.matmul(at_ps[:, qt, :], exj, v_sb[:, t, h, :],
                                         start=(t == 0 and qt % 8 == 0),
                                         stop=(t == qt), skip_group_check=True)
                        nc.tensor.matmul(den_ps[:, qt, :], exj, ones_sb[:],
                                         start=(t == 0 and qt == 0),
                                         stop=(t == qt), skip_group_check=True)
                    off += n
                if t % 4 == 3 and t >= 4:
                    finish_group(h, t // 4)

        for h in range(4):
            attention(h)


_CACHE = {}


def _build():
    if "nc" in _CACHE:
        return _CACHE["nc"], _CACHE["aps"]
    nc = bacc.Bacc("TRN2", target_bir_lowering=False, debug=False,
                   enable_asserts=False, num_devices=N_CORES)
    aps = {
        "xT8": nc.dram_tensor("xT8", [128, 4, 2, S], FP8, kind="ExternalInput").ap(),
        "wq8": nc.dram_tensor("wq8", [128, 4, 2, 2, 128], FP8, kind="ExternalInput").ap(),
        "wk8": nc.dram_tensor("wk8", [128, 4, 2, 2, 128], FP8, kind="ExternalInput").ap(),
        "wq8s": nc.dram_tensor("wq8s", [128, 4, 2, 2, 128], FP8, kind="ExternalInput").ap(),
        "wk8s": nc.dram_tensor("wk8s", [128, 4, 2, 2, 128], FP8, kind="ExternalInput").ap(),
        "wv8": nc.dram_tensor("wv8", [128, 4, 2, DPC], FP8, kind="ExternalInput").ap(),
        "wo8": nc.dram_tensor("wo8", [128, 2, D], FP8, kind="ExternalInput").ap(),
        "cosT": nc.dram_tensor("cosT", [128, 2, S], BF16, kind="ExternalInput").ap(),
        "sinT": nc.dram_tensor("sinT", [128, 2, S], BF16, kind="ExternalInput").ap(),
        "ident": nc.dram_tensor("ident", [128, 128], BF16, kind="ExternalInput").ap(),
        "dmask": nc.dram_tensor("dmask", [128, 128], BF16, kind="ExternalInput").ap(),
        "xT_hi": nc.dram_tensor("xT_hi", [128, 8, QHI], BF16, kind="ExternalInput").ap(),
        "wq_hi": nc.dram_tensor("wq_hi", [128, 8, 2, 128], BF16, kind="ExternalInput").ap(),
        "wk_hi": nc.dram_tensor("wk_hi", [128, 8, 2, 128], BF16, kind="ExternalInput").ap(),
        "wv_hi": nc.dram_tensor("wv_hi", [128, 8, DPC], BF16, kind="ExternalInput").ap(),
        "wo_hi": nc.dram_tensor("wo_hi", [128, 2, D], BF16, kind="ExternalInput").ap(),
        "out": nc.dram_tensor("out", [S, D], BF16, kind="ExternalOutput").ap(),
    }
    with tile.TileContext(nc) as tc:
        _emit(tc, aps)
    nc.compile()
    _CACHE["nc"], _CACHE["aps"] = nc, aps
    return nc, aps


def _host_tables():
    """cos/sin tables [128, 2(mt), S] bf16 in the permuted row order, /16."""
    p = np.arange(128)
    pos = np.arange(S, dtype=np.float64)
    cosT = np.empty((128, 2, S), np.float32)
    sinT = np.empty((128, 2, S), np.float32)
    for mt in range(2):
        dk_idx = 32 * mt + (p % 32)              # [128]
        i = dk_idx // 2
        freq = THETA ** (-2.0 * i / DK)          # [128]
        ang = pos[None, :] * freq[:, None]       # [128, S]
        cosT[:, mt, :] = np.cos(ang) / WS
        sgn = np.where(dk_idx % 2 == 0, -1.0, 1.0)
        sinT[:, mt, :] = sgn[:, None] * np.sin(ang) / WS
    return (cosT.astype(ml_dtypes.bfloat16), sinT.astype(ml_dtypes.bfloat16))


def _perm(swap):
    """Row permutation: (mt*128+p) -> local q-dim (64*h + dk)."""
    p = np.arange(128)
    out = []
    for mt in range(2):
        dk = 32 * mt + (p % 32)
        if swap:
            dk = dk ^ 1
        out.append(64 * (p // 32) + dk)
    return np.concatenate(out)                   # [256]


def _pack_w(Wc, swap):
    """Wc [256, 1024] (rows = local q-dims) -> [128, 4, 2, 2, 128] fp8."""
    arr = (Wc[_perm(swap)] * WS).reshape(2, 128, 4, 2, 128)  # [mt, m, kt2, s, p]
    return np.ascontiguousarray(
        arr.transpose(4, 2, 3, 0, 1)).astype(ml_dtypes.float8_e4m3)


def _pack_w_hi(Wc):
    """Wc [256, 1024] -> [128, 8, 2, 128] bf16 (x16, perm rows, kt=128)."""
    arr = (Wc[_perm(False)] * WS).reshape(2, 128, 8, 128)  # [mt, m, kt, p]
    return np.ascontiguousarray(arr.transpose(3, 2, 0, 1)).astype(ml_dtypes.bfloat16)


def make_in_maps(x, Wq, Wk, Wv, Wo):
    cosT, sinT = _host_tables()
    ident = np.eye(128, dtype=ml_dtypes.bfloat16)
    dmask = np.triu(np.ones((128, 128), ml_dtypes.bfloat16))  # keep sq >= sk
    xT8 = []
    for b in range(B):
        xr = np.ascontiguousarray(
            x[b].T.reshape(4, 2, 128, S).transpose(2, 0, 1, 3))
        xT8.append(xr.astype(ml_dtypes.float8_e4m3))
    maps = []
    for c in range(N_CORES):
        b, g = c // 4, c % 4
        rows = slice(DPC * g, DPC * (g + 1))
        wv = np.ascontiguousarray(
            (Wv[rows, :].T * WS).reshape(4, 2, 128, DPC).transpose(2, 0, 1, 3))
        wo = np.ascontiguousarray(
            (Wo[:, rows].T * WS).reshape(2, 128, D).transpose(1, 0, 2))
        wv_hi = np.ascontiguousarray(
            (Wv[rows, :].T * WS).reshape(8, 128, DPC).transpose(1, 0, 2)
        ).astype(ml_dtypes.bfloat16)
        wo_hi = np.ascontiguousarray(
            (Wo[:, rows].T * WS).reshape(2, 128, D).transpose(1, 0, 2)
        ).astype(ml_dtypes.bfloat16)
        xhi = np.ascontiguousarray(
            x[b][0:QHI].T.reshape(8, 128, QHI).transpose(1, 0, 2)
        ).astype(ml_dtypes.bfloat16)
        maps.append({
            "xT8": xT8[b],
            "xT_hi": xhi,
            "wq_hi": _pack_w_hi(Wq[rows, :]),
            "wk_hi": _pack_w_hi(Wk[rows, :]),
            "wv_hi": wv_hi,
            "wo_hi": wo_hi,
            "wq8": _pack_w(Wq[rows, :], False),
            "wk8": _pack_w(Wk[rows, :], False),
            "wq8s": _pack_w(Wq[rows, :], True),
            "wk8s": _pack_w(Wk[rows, :], True),
            "wv8": wv.astype(ml_dtypes.float8_e4m3),
            "wo8": wo.astype(ml_dtypes.float8_e4m3),
            "cosT": cosT,
            "sinT": sinT,
            "ident": ident,
            "dmask": dmask,
        })
    return maps


def kernel(x, Wq, Wk, Wv, Wo, _trace=False, _tmpdir=None):
    x, Wq, Wk, Wv, Wo = (np.asarray(a, dtype=np.float32)
                         for a in (x, Wq, Wk, Wv, Wo))
    nc, _ = _build()
    maps = make_in_maps(x, Wq, Wk, Wv, Wo)
    res = run_bass_kernel_spmd(nc, maps, core_ids=list(range(N_CORES)),
                               trace=_trace, tmpdir=_tmpdir)
    out = np.zeros((B, S, D), np.float32)
    for c in range(N_CORES):
        out[c // 4] += res.results[c]["out"].astype(np.float32)
    if _trace:
        kernel.last_results = res
    return out
